# revision 24
# baseline (speedup 1.0000x reference)
"""Adaptive average pool 2D (64x64 -> 6x6) on 8 TRN2 NeuronCores.

Input  x: (16, 2048, 64, 64) f32
Output  : (16, 2048, 6, 6) f32

Sharding: data-parallel over the batch dim (2 batches per core).
Per-core kernel: 32 tiles of [128 channels (partitions), 4096 (h*w)].
Stage 1 reduces W (64 -> 6 bins), stage 2 reduces H, then scale by
1/(bin_h*bin_w).  Adaptive bin starts come in stride-32 pairs
({10,42}, {0,32}, {21,53}) so each stage is 3 strided DVE reduces.
"""

import numpy as np

import concourse.bass as bass
import concourse.bacc as bacc
import concourse.mybir as mybir
from concourse.ap import AP
from concourse.tile import TileContext
from concourse.bass_utils import run_bass_kernel_spmd

# ---------------------------------------------------------------- problem dims
N, C, H_IN, W_IN = 16, 2048, 64, 64
H_OUT = W_OUT = 6
N_CORES = 8
N_PER_CORE = N // N_CORES          # 2
CBLK = 128
ROWS = N_PER_CORE * C              # 4096 (n,c) rows per core
NTILES = ROWS // CBLK              # 32
HW = H_IN * W_IN                   # 4096
F32 = mybir.dt.float32

# Adaptive pool bin edges (floor/ceil rule, same for H and W since 64->6).
_STARTS = [0, 10, 21, 32, 42, 53]
_SIZES = [11, 12, 11, 11, 12, 11]
# Reduce groups: (bin indices pair, start of first, bin size).
# Second member of each pair starts +32 elements later.
_GROUPS = [((1, 4), 10, 12), ((0, 3), 0, 11), ((2, 5), 21, 11)]


def _ap(t: bass.AP, off: int, dims) -> AP:
    """Custom strided AP on a pool tile (keeps the tile's partition dim)."""
    assert t.offset == 0
    return AP(t.tensor, off, [list(t.ap[0])] + [list(d) for d in dims])


def build_nc(io_bufs: int = 6, mid_bufs: int = 4,
             load_split: int = 4) -> bass.Bass:
    # Bacc (not raw Bass): its compile() runs generate_event_semaphores,
    # which splits multi-sem waits to satisfy the TRN2 1-wait-per-
    # instruction ISA limit.
    nc = bacc.Bacc("TRN2", target_bir_lowering=False, debug=False,
                   num_devices=N_CORES)
    x = nc.dram_tensor("x", [ROWS, HW], F32, kind="ExternalInput")
    out = nc.dram_tensor("out", [ROWS, H_OUT * W_OUT], F32,
                         kind="ExternalOutput")

    with TileContext(nc) as tc:
        with tc.tile_pool(name="io", bufs=io_bufs) as io_pool, \
             tc.tile_pool(name="mid", bufs=mid_bufs) as mid_pool:
            for t in range(NTILES):
                row0 = t * CBLK
                xt = io_pool.tile([CBLK, HW], F32, name="xt", tag="xt")
                # Split each load across the SP (HWDGE) and Pool (SWDGE)
                # sequencers: a single issue path sustains only ~300 GB/s,
                # the halves transfer in parallel, and neither engine has
                # compute work that could stall its in-order stream.
                step = HW // load_split
                for p in range(load_split):
                    load_eng = nc.sync if p % 2 == 0 else nc.gpsimd
                    load_eng.dma_start(
                        out=xt[:, p * step:(p + 1) * step],
                        in_=x[row0:row0 + CBLK, p * step:(p + 1) * step])

                # yt: stage-1 result, layout [c, (j, h)] j-major (j step 64).
                yt = mid_pool.tile([CBLK, W_OUT * H_IN], F32, name="yt",
                                   tag="yt")
                for (j0, j1), w0, wsz in _GROUPS:
                    # in: [c, h(64, step 64), jpair(2, step 32), w(wsz, 1)]
                    src = _ap(xt, w0, [[W_IN, H_IN], [32, 2], [1, wsz]])
                    # out: [c, h(64, step 1), jpair(2, step 192)]
                    dst = _ap(yt, j0 * H_IN, [[1, H_IN], [192, 2]])
                    nc.vector.reduce_sum(dst, src, axis=mybir.AxisListType.X)

                # ot: final [c, 36] i-major (col = i*6 + j).
                ot = mid_pool.tile([CBLK, H_OUT * W_OUT], F32, name="ot",
                                   tag="ot")
                for (i0, i1), h0, hsz in _GROUPS:
                    # in on yt: [c, j(6, step 64), ipair(2, step 32), h(hsz)]
                    src = _ap(yt, h0, [[H_IN, W_OUT], [32, 2], [1, hsz]])
                    # out: [c, j(6, step 1), ipair(2, step 18)]
                    dst = _ap(ot, i0 * W_OUT, [[1, W_OUT], [18, 2]])
                    nc.vector.reduce_sum(dst, src, axis=mybir.AxisListType.X)

                # Mean scale 1/(h_size*w_size): 9 stride-regular 2x2 (i, j)
                # pair-groups, in-place on the otherwise-idle ACT engine so
                # the DVE does only the reduces.
                for (i0, _i1), _h0, hsz in _GROUPS:
                    for (j0, _j1), _w0, wsz in _GROUPS:
                        dst = _ap(ot, i0 * W_OUT + j0, [[18, 2], [3, 2]])
                        nc.scalar.mul(dst, dst, 1.0 / (hsz * wsz))

                # Store via the ACT sequencer (HWDGE): it directly follows
                # the scale muls in ACT program order, and keeps the SP
                # sequencer free to issue loads far ahead (a store waiting
                # on compute would otherwise stall SP's in-order stream).
                nc.scalar.dma_start(out=out[row0:row0 + CBLK, :], in_=ot)
    nc.compile()
    return nc


_NC_CACHE = None


def _get_nc() -> bass.Bass:
    global _NC_CACHE
    if _NC_CACHE is None:
        _NC_CACHE = build_nc()
    return _NC_CACHE


def run(x: np.ndarray, **spmd_kwargs):
    """Shard, run on 8 cores, gather.  Returns (output, BassKernelResults)."""
    x = np.ascontiguousarray(np.asarray(x), dtype=np.float32)
    assert x.shape == (N, C, H_IN, W_IN)
    in_maps = [
        {"x": x[i * N_PER_CORE:(i + 1) * N_PER_CORE].reshape(ROWS, HW)}
        for i in range(N_CORES)
    ]
    res = run_bass_kernel_spmd(_get_nc(), in_maps, list(range(N_CORES)),
                               **spmd_kwargs)
    out = np.concatenate(
        [res.results[i]["out"].reshape(N_PER_CORE, C, H_OUT, W_OUT)
         for i in range(N_CORES)], axis=0)
    return out, res


def kernel(x: np.ndarray) -> np.ndarray:
    out, _ = run(x)
    return out
